# revision 36
# baseline (speedup 1.0000x reference)
"""Trainium2 Bass kernel for the annular photonic transfer-matrix reflectance
sweep (W=2097152 wavelengths, L=6 layers), data-parallel over 8 NeuronCores.

Formulation (validated host-side to rel_l2 ~1.6e-3 vs the jax reference):
- Each shell's 2x2 transfer matrix entries are written as
      a = Ca(t)cosD - Sa(t)sinD,   b = (Cb sinD + Sb cosD)/p,
      c = (Cc sinD + Sc cosD)*p,   d = Cd cosD - Sd sinD,
  with t = omega*sqrt(eps), D = (r1-r0)*t, p = sqrt(eps).  The 8 smooth
  C/S product functions (Bessel amplitude/phase combinations at x0=t*r0,
  x1=t*r1) are fitted per shell by QUADRATICS in v = 1/t at build time
  (the reference's own y1 has a ~1e-3 jump at x=8, so degree>2 buys
  nothing).  Each quadratic is evaluated as A*(v+B)^2 + C: one ACT Square
  (free affine bias) + one DVE tensor_scalar.
- sinD/cosD: reduce t mod pi/c in ONE scalar_tensor_tensor (k<=14 so the
  single-constant reduction error is ~2e-6 rad), then ACT Sin with the
  *c fold in its free scale; the dropped (-1)^k sign scales the whole
  shell matrix and cancels in R = |num/den|^2.
- All smooth math runs in fp16 (DVE 2x/4x modes); t and the range
  reduction stay fp32.  Boundary C(z) factors are fitted the same way.
- mu == 1 per the spec; a numpy fallback guards other inputs.
"""
import numpy as np

import bass_rust
import concourse.bass as bass
import concourse.tile as tile
from concourse import mybir
from concourse.vector_clock import ScopedClock

F32 = mybir.dt.float32
F16 = mybir.dt.float16
AL = mybir.AluOpType
AF = mybir.ActivationFunctionType

W = 2097152
L = 6
NCORES = 8
P = 128
WS = W // NCORES          # 262144 elements per core
FT = WS // P              # 2048 free dim per core
FC = 1024                 # free-dim chunk per pass (2 passes)

PI = float(np.pi)
TWO_OVER_PI = 0.636619772
MAGIC = 8388608.0

# ---- NR Bessel coefficients (reference's own formulas, fp64 host eval) ------

J0_NUM = [-184.9052456, 77392.33017, -11214424.18, 651619640.7,
          -13362590354.0, 57568490574.0]
J0_DEN = [1.0, 267.8532712, 59272.64853, 9494680.718,
          1029532985.0, 57568490411.0]
J1_NUM = [-30.16036606, 15704.48260, -2972611.439, 242396853.1,
          -7895059235.0, 72362614232.0]
J1_DEN = [1.0, 376.9991397, 99447.43394, 18583304.74,
          2300535178.0, 144725228442.0]
Y0_NUM = [228.4622733, -86327.92757, 10879881.29, -512359803.6,
          7062834065.0, -2957821389.0]
Y0_DEN = [1.0, 226.1030244, 47447.26470, 7189466.438,
          745249964.8, 40076544269.0]
Y1_NUM = [8.511937935e4, -4.237922726e7, 7.349264551e9,
          -5.153438139e11, 1.275274390e13, -4.900604943e13]
Y1_DEN = [1.0, 3.549632885e3, 1.020426050e6, 2.245904002e8,
          3.733650367e10, 4.244419664e12, 2.499580570e14]
P0C = [0.2093887211e-6, -0.2073370639e-5, 0.2734510407e-4,
       -0.1098628627e-2, 1.0]
Q0C = [-0.934935152e-7, 0.7621095161e-6, -0.6911147651e-5,
       0.1430488765e-3, -0.1562499995e-1]
P1C = [-0.240337019e-6, 0.2457520174e-5, -0.3516396496e-4,
       0.183105e-2, 1.0]
Q1C = [0.105787412e-6, -0.88228987e-6, 0.8449199096e-5,
       -0.2002690873e-3, 0.04687499995]


def _hpoly(y, c):
    acc = np.full_like(y, c[0])
    for v in c[1:]:
        acc = acc * y + v
    return acc


def _j0(x):
    y = x * x
    small = _hpoly(y, J0_NUM) / _hpoly(y, J0_DEN)
    z = 8.0 / x
    y2 = z * z
    xx = x - 0.785398164
    big = np.sqrt(TWO_OVER_PI / x) * (np.cos(xx) * _hpoly(y2, P0C)
                                      - z * np.sin(xx) * _hpoly(y2, Q0C))
    return np.where(x < 8.0, small, big)


def _j1(x):
    y = x * x
    small = x * _hpoly(y, J1_NUM) / _hpoly(y, J1_DEN)
    z = 8.0 / x
    y2 = z * z
    xx = x - 2.356194491
    big = np.sqrt(TWO_OVER_PI / x) * (np.cos(xx) * _hpoly(y2, P1C)
                                      - z * np.sin(xx) * _hpoly(y2, Q1C))
    return np.where(x < 8.0, small, big)


def _y0(x):
    y = x * x
    small = _hpoly(y, Y0_NUM) / _hpoly(y, Y0_DEN) \
        + TWO_OVER_PI * _j0(x) * np.log(x)
    z = 8.0 / x
    y2 = z * z
    xx = x - 0.785398164
    big = np.sqrt(TWO_OVER_PI / x) * (np.sin(xx) * _hpoly(y2, P0C)
                                      + z * np.cos(xx) * _hpoly(y2, Q0C))
    return np.where(x < 8.0, small, big)


def _y1(x):
    y = x * x
    small = x * _hpoly(y, Y1_NUM) / _hpoly(y, Y1_DEN) \
        + TWO_OVER_PI * (_j1(x) * np.log(x) - 1.0 / x)
    z = 8.0 / x
    y2 = z * z
    xx = x - 2.356194491
    big = np.sqrt(TWO_OVER_PI / x) * (np.sin(xx) * _hpoly(y2, P1C)
                                      + z * np.cos(xx) * _hpoly(y2, Q1C))
    return np.where(x < 8.0, small, big)


def _mphi(x):
    amp = np.sqrt(PI * x / 2.0)
    j0n, y0n = _j0(x) * amp, _y0(x) * amp
    j1n, y1n = _j1(x) * amp, _y1(x) * amp
    psi = x - PI / 4
    m0 = np.hypot(j0n, y0n)
    ph0 = np.angle(np.exp(1j * (np.arctan2(y0n, j0n) - psi)))
    m1 = np.hypot(j1n, y1n)
    ph1 = np.angle(np.exp(1j * (np.arctan2(j1n, -y1n) - psi)))
    return m0, ph0, m1, ph1


def _shell_funcs(t, r0, r1):
    m0a, f0a, m1a, f1a = _mphi(t * r0)
    m0b, f0b, m1b, f1b = _mphi(t * r1)
    return (m1a * m0b * np.cos(f0b - f1a), m1a * m0b * np.sin(f0b - f1a),
            m0a * m0b * np.cos(f0b - f0a), m0a * m0b * np.sin(f0b - f0a),
            m1a * m1b * np.cos(f1b - f1a), m1a * m1b * np.sin(f1b - f1a),
            m0a * m1b * np.cos(f1b - f0a), m0a * m1b * np.sin(f1b - f0a))


def _bound_funcs(t, r):
    m0, ph0, m1, ph1 = _mphi(t * r)
    d = ph1 - ph0
    rm = m1 / m0
    return rm * np.sin(d), rm * np.cos(d)


LIN_TOL = 2.5e-3


def _fit_quad(f, lo, hi, n=3000):
    """Fit f on [lo,hi]: linear if it reaches LIN_TOL, else quadratic in
    square-form ("quad", A, B, C) for A*(v+B)^2 + C."""
    k = np.arange(n)
    x = lo + (hi - lo) * 0.5 * (1 - np.cos(np.pi * (k + 0.5) / n))
    y = f(x)
    ch1 = np.polynomial.chebyshev.Chebyshev.fit(x, y, 1, domain=[lo, hi])
    if np.abs(ch1(x) - y).max() < LIN_TOL:
        c1, c0 = 0.0, 0.0
        co = ch1.convert(kind=np.polynomial.Polynomial).coef
        c2 = co[0]
        c1 = co[1] if len(co) > 1 else 0.0
        return ("lin", float(c1), float(c2))
    ch = np.polynomial.chebyshev.Chebyshev.fit(x, y, 2, domain=[lo, hi])
    c2, c1, c0 = ch.convert(kind=np.polynomial.Polynomial).coef
    if abs(c0) < 1e-9:
        c0 = 1e-9 if c0 >= 0 else -1e-9
    return ("quad", float(c0), float(c1 / (2 * c0)),
            float(c2 - c1 * c1 / (4 * c0)))


def _build_fits(rho, tlo, thi):
    """rho: [L,2] float64; tlo/thi: per-layer t bounds. Returns dict."""
    fits = {}
    for l in range(1, L - 1):
        lo, hi = 1.0 / thi[l], 1.0 / tlo[l]
        r0, r1 = float(rho[l, 0]), float(rho[l, 1])
        for i, nm in enumerate(["Ca", "Sa", "Cb", "Sb", "Cc", "Sc",
                                "Cd", "Sd"]):
            fits[(l, nm)] = _fit_quad(
                lambda v, i=i: _shell_funcs(1.0 / v, r0, r1)[i], lo, hi)
    for (l, rr, pre) in [(0, float(rho[0, 1]), "b0"),
                         (L - 1, float(rho[L - 1, 0]), "b1")]:
        lo, hi = 1.0 / thi[l], 1.0 / tlo[l]
        for i, sfx in enumerate(["re", "im"]):
            fits[(l, pre + sfx)] = _fit_quad(
                lambda v, i=i: _bound_funcs(1.0 / v, rr)[i], lo, hi)
    return fits


# ---- walrus 1-sync-wait-per-instruction workaround --------------------------
_MAXW = 1


def _split_waits(nc):
    for f in nc.m.functions:
        for bb in f.blocks:
            arr = list(bb.instructions)
            out = []
            changed = False
            for mi in arr:
                si = mi.sync_info
                waits = list(si.on_wait) if si is not None and si.on_wait else []
                if len(waits) > _MAXW:
                    changed = True
                    upd = list(si.on_update) if si is not None and si.on_update \
                        else []
                    rest = waits[_MAXW:]
                    for i in range(0, len(rest), _MAXW):
                        ev = nc.engines[mi.engine].nop()
                        cur = nc.cur_bb.bb
                        cur.instructions = [
                            x for x in cur.instructions if x.name != ev.ins.name
                        ]
                        ev.ins.sync_info = bass_rust.SyncInfo(
                            on_wait=rest[i:i + _MAXW], on_update=[])
                        out.append(ev.ins)
                    mi.sync_info = bass_rust.SyncInfo(on_wait=waits[:_MAXW],
                                                      on_update=upd)
                out.append(mi)
            if changed:
                bb.instructions = out


def _patched_drain_and_barrier(self, tick_clock, wait_clock):
    nc = self.nc
    drain_inst = nc.sync.drain()
    wait_clock.add_sem_waits(
        drain_inst.ins, ScopedClock({None: tick_clock.global_clock})
    )
    nc.all_engine_barrier()
    assert self.sems is not None
    popped = nc._tile_sem_poison_stack.pop()
    assert popped is self._sem_poison
    nc.clear_and_free_semaphores(list(self.sems.allocated().values()))
    nc.all_engine_barrier()


tile.TileContext._drain_and_barrier = _patched_drain_and_barrier


def _register_const(nc, *values):
    for v in values:
        v = float(v)
        if (F32, v) in nc.const_aps.aps:
            continue
        t = nc.alloc_sbuf_tensor(f"const-f32-{v}", [128, 1], F32)
        nc.gpsimd.memset(t.ap(), v)
        nc.const_aps.aps[(F32, v)] = t.ap()
    nc.all_engine_barrier()


# ---- kernel emitter ---------------------------------------------------------

SHELL_FN = ["Ca", "Sa", "Cb", "Sb", "Cc", "Sc", "Cd", "Sd"]


def build(rho64, fits):
    nc = bass.Bass()
    biases = {float(np.float32(v[2])) for v in fits.values() if v[0] == "quad"}
    _register_const(nc, 0.0, PI / 2, *sorted(biases))

    om_d = nc.declare_dram_parameter("omega", [P, FT], F32, isOutput=False)
    ep_d = nc.declare_dram_parameter("eps", [L, P, FT], F32, isOutput=False)
    out_d = nc.declare_dram_parameter("out", [P, FT], F32, isOutput=True)

    with tile.TileContext(nc) as tc:
        with tc.tile_pool(name="work", bufs=1) as pool:
            n = [0]

            def mk(dt, tag, bufs):
                n[0] += 1
                return pool.tile([P, FC], dt, name=f"t{n[0]}", tag=tag,
                                 bufs=bufs)

            def w32(tag="g32", bufs=5):
                return mk(F32, tag, bufs)

            def w16(tag="g16", bufs=14):
                return mk(F16, tag, bufs)

            def act(out, in_, fn, bias=0.0, scale=1.0):
                nc.scalar.activation(out[:], in_[:], fn, float(bias),
                                     float(scale))
                return out

            def vts(out, a, s1, s2=None, op0="mult", op1="add"):
                if s2 is None:
                    nc.vector.tensor_scalar(out[:], a[:], float(s1), None,
                                            AL[op0])
                else:
                    nc.vector.tensor_scalar(out[:], a[:], float(s1),
                                            float(s2), AL[op0], AL[op1])
                return out

            def tt(out, a, b, op):
                nc.vector.tensor_tensor(out[:], a[:], b[:], AL[op])
                return out

            def stt(out, a, s, b, op0="mult", op1="add"):
                nc.vector.scalar_tensor_tensor(out[:], a[:], float(s), b[:],
                                               AL[op0], AL[op1])
                return out

            def poly16(fit, v16l):
                kind = fit[0]
                if kind == "lin":
                    return act(w16(), v16l, AF.Copy, fit[2], fit[1])
                _, A, B, C = fit
                B = float(np.float32(B))
                if abs(B) < 2.0:
                    q = act(w16(), v16l, AF.Square, B)
                    return vts(w16(), q, A, C)
                q = act(w32(), v16l, AF.Square, B)
                return act(w16(), q, AF.Copy, C, A)

            def chunk(ci, pre_tail=None):
                sl = slice(ci * FC, (ci + 1) * FC)
                omega = w32(tag="om", bufs=2)
                nc.sync.dma_start(omega[:], om_d[:, sl])
                ln_om = act(w32(tag="lnom", bufs=1), omega, AF.Ln)
                t16, v16, t_ = {}, {}, {}

                def layerA(l):
                    e = w32(tag="eps", bufs=2)
                    nc.sync.dma_start(e[:], ep_d[l, :, sl])
                    lne = act(w32(tag="sq", bufs=3), e, AF.Ln)
                    ln_t = stt(w32(tag="ln", bufs=2), lne, 0.5, ln_om)
                    t_[l] = act(w32(tag="t", bufs=4), ln_t, AF.Exp)
                    t16[l] = vts(w16(tag="t16", bufs=8), t_[l], 1.0)
                    v16[l] = act(w16(tag="v16", bufs=8), ln_t, AF.Exp,
                                 0.0, -1.0)

                def boundary(l, pre):
                    cre = poly16(fits[(l, pre + "re")], v16[l])
                    cim = poly16(fits[(l, pre + "im")], v16[l])
                    ur = tt(w16(tag="bnd", bufs=10), t16[l], cre, "mult")
                    ui = tt(w16(tag="bnd", bufs=10), t16[l], cim, "mult")
                    return ur, ui

                def shell(l):
                    r0 = float(rho64[l, 0])
                    r1 = float(rho64[l, 1])
                    c = float(np.float32(np.float64(r1) - np.float64(r0)))
                    cpi = float(np.float32(np.float64(c) / np.pi))
                    pic = float(np.float32(np.pi / np.float64(c)))
                    tr = vts(w32(), t_[l], cpi)
                    kf = vts(w32(), tr, MAGIC, MAGIC, "add", "subtract")
                    xr = stt(w32(), kf, -pic, t_[l])
                    SD = act(w16(tag="sdcd", bufs=6), xr, AF.Sin, 0.0, c)
                    CD = act(w16(tag="sdcd", bufs=6), xr, AF.Sin,
                             PI / 2, -c)
                    Pv = {nm: poly16(fits[(l, nm)], v16[l])
                          for nm in SHELL_FN}
                    TCa = tt(w16(), Pv["Ca"], CD, "mult")
                    TSa = tt(w16(), Pv["Sa"], SD, "mult")
                    TCb = tt(w16(), Pv["Cb"], SD, "mult")
                    TSb = tt(w16(), Pv["Sb"], CD, "mult")
                    TCc = tt(w16(), Pv["Cc"], SD, "mult")
                    TSc = tt(w16(), Pv["Sc"], CD, "mult")
                    TCd = tt(w16(), Pv["Cd"], CD, "mult")
                    TSd = tt(w16(), Pv["Sd"], SD, "mult")
                    a = tt(w16(tag="mm", bufs=16), TCa, TSa, "subtract")
                    beta = tt(w16(), TCb, TSb, "add")
                    gam = tt(w16(), TCc, TSc, "add")
                    d = tt(w16(tag="mm", bufs=16), TCd, TSd, "subtract")
                    b = tt(w16(tag="mm", bufs=16), beta, v16[l], "mult")
                    cc = tt(w16(tag="mm", bufs=16), gam, t16[l], "mult")
                    return a, b, cc, d

                def join(Mx, My):
                    a1, b1, c1, d1 = Mx
                    a2, b2, c2, d2 = My
                    A = tt(w16(tag="mm", bufs=16),
                           tt(w16(), a1, a2, "mult"),
                           tt(w16(), b1, c2, "mult"), "subtract")
                    Bq = tt(w16(tag="mm", bufs=16),
                            tt(w16(), a1, b2, "mult"),
                            tt(w16(), b1, d2, "mult"), "add")
                    C = tt(w16(tag="mm", bufs=16),
                           tt(w16(), c1, a2, "mult"),
                           tt(w16(), d1, c2, "mult"), "add")
                    D = tt(w16(tag="mm", bufs=16),
                           tt(w16(), d1, d2, "mult"),
                           tt(w16(), c1, b2, "mult"), "subtract")
                    return A, Bq, C, D

                layerA(0)
                layerA(L - 1)
                if pre_tail is not None:
                    pre_tail()
                layerA(1)
                u0 = boundary(0, "b0")
                u1 = boundary(L - 1, "b1")
                M = shell(1)
                layerA(2)
                M = join(M, shell(2))
                layerA(3)
                M = join(M, shell(3))
                layerA(4)
                M = join(M, shell(4))
                return dict(sl=sl, u0=u0, u1=u1, M=M)

            def tail(S):
                A, B, C, D = S["M"]
                ur0, ui0 = S["u0"]
                vr0, vi0 = S["u1"]
                Q = tt(w16(), ui0, B, "mult")
                er = tt(w16(), D, tt(w16(), ur0, B, "mult"), "add")
                T1 = tt(w16(), vi0, Q, "mult")
                T2 = tt(w16(), vr0, er, "mult")
                T3 = tt(w16(), vr0, Q, "mult")
                T4 = tt(w16(), vi0, er, "mult")
                aAr = tt(w16(), ur0, A, "mult")
                aAi = tt(w16(), ui0, A, "mult")
                b0 = tt(w16(), C, aAr, "subtract")
                b1 = tt(w16(), b0, T2, "add")
                Nr = tt(w16(), b1, T1, "subtract")
                Dr = tt(w16(), b1, T1, "add")
                c0_ = tt(w16(), aAi, T3, "subtract")
                Ni = tt(w16(), c0_, T4, "subtract")
                Di = tt(w16(), c0_, T4, "add")
                SN = tt(w32(), act(w32(), Nr, AF.Square),
                        act(w32(), Ni, AF.Square), "add")
                SDn = tt(w32(), act(w32(), Dr, AF.Square),
                         act(w32(), Di, AF.Square), "add")
                lnD = act(w32(), SDn, AF.Ln)
                rec = act(w32(), lnD, AF.Exp, 0.0, -1.0)
                R = tt(w32(), SN, rec, "mult")
                nc.sync.dma_start(out_d[:, S["sl"]], R[:])

            S0 = chunk(0)
            tail(S0)
            S1 = chunk(1)
            tail(S1)
    _split_waits(nc)
    return nc


# ---- host-side entry --------------------------------------------------------

_CACHE = {}
TRACE = False
LAST_RESULT = None


def _numpy_ref(omega, eps, mu, rho):
    """Exact reference math in numpy (fallback for mu != 1)."""
    omega = omega.astype(np.float64)
    eps = eps.astype(np.float64)
    mu = mu.astype(np.float64)
    rho = rho.astype(np.float64)
    k = omega[None, :] * np.sqrt(eps * mu)
    p = np.sqrt(eps / mu)

    def tmat(kl, pl, r0, r1):
        x0, x1 = kl * r0, kl * r1
        j_a, y_a = _j0(x0), _y0(x0)
        j_b, y_b = _j0(x1), _y0(x1)
        jd_a, yd_a = -_j1(x0), -_y1(x0)
        jd_b, yd_b = -_j1(x1), -_y1(x1)
        pref = (PI / 2) * x0
        m00 = pref * (yd_a * j_b - jd_a * y_b)
        m01 = (1j / pl) * pref * (j_a * y_b - y_a * j_b)
        m10 = (-1j * pl) * pref * (yd_a * jd_b - jd_a * yd_b)
        m11 = pref * (j_a * yd_b - y_a * jd_b)
        return m00 + 0j, m01, m10, m11 + 0j

    M00, M01, M10, M11 = tmat(k[1], p[1], rho[1, 0], rho[1, 1])
    for l in range(2, L - 1):
        a, b, c, d = tmat(k[l], p[l], rho[l, 0], rho[l, 1])
        M00, M01, M10, M11 = (M00 * a + M01 * c, M00 * b + M01 * d,
                              M10 * a + M11 * c, M10 * b + M11 * d)

    def cfacs(z):
        j0v, j1v, y0v, y1v = _j0(z), _j1(z), _y0(z), _y1(z)
        c1 = -(j1v + 1j * y1v) / (j0v + 1j * y0v)
        c2 = -(j1v - 1j * y1v) / (j0v - 1j * y0v)
        return c1, c2

    c0_1, c0_2 = cfacs(k[0] * rho[0, 1])
    _, c1_2 = cfacs(k[L - 1] * rho[L - 1, 0])
    p0, p1 = p[0], p[L - 1]
    num = M10 + 1j * p0 * c0_2 * M00 \
        - 1j * p1 * c1_2 * (M11 + 1j * p0 * c0_2 * M01)
    den = -1j * p0 * c0_1 * M00 - M10 \
        - 1j * p1 * c1_2 * (-1j * p0 * c0_1 * M01 - M11)
    r = num / den
    return (r * np.conj(r)).real.astype(np.float32)


def kernel(omega, eps, mu, rho):
    from concourse.bass_utils import run_bass_kernel_spmd

    omega = np.ascontiguousarray(omega, dtype=np.float32)
    eps = np.ascontiguousarray(eps, dtype=np.float32)
    mu = np.ascontiguousarray(mu, dtype=np.float32)
    rho = np.asarray(rho, dtype=np.float32)
    assert omega.shape == (W,) and eps.shape == (L, W)

    if not bool(np.all(mu == 1.0)):
        return _numpy_ref(omega, eps, mu, rho)

    rho64 = rho.astype(np.float64)
    om_lo, om_hi = float(omega.min()), float(omega.max())
    e_lo = eps.min(axis=1).astype(np.float64)
    e_hi = eps.max(axis=1).astype(np.float64)
    tlo = om_lo * np.sqrt(e_lo) * 0.999
    thi = om_hi * np.sqrt(e_hi) * 1.001

    key = (rho.tobytes(),
           tuple(np.round(tlo, 3).tolist()), tuple(np.round(thi, 3).tolist()))
    if key not in _CACHE:
        fits = _build_fits(rho64, tlo, thi)
        _CACHE[key] = build(rho64, fits)
    nc = _CACHE[key]

    in_maps = []
    for i in range(NCORES):
        sl = slice(i * WS, (i + 1) * WS)
        in_maps.append({"omega": omega[sl].reshape(P, FT),
                        "eps": eps[:, sl].reshape(L, P, FT)})

    res = run_bass_kernel_spmd(nc, in_maps, core_ids=list(range(NCORES)),
                               trace=TRACE)
    global LAST_RESULT
    LAST_RESULT = res
    out = np.empty((W,), dtype=np.float32)
    for i in range(NCORES):
        out[i * WS:(i + 1) * WS] = res.results[i]["out"].reshape(WS)
    return out


# revision 37
# speedup vs baseline: 1.0068x; 1.0068x over previous
"""Trainium2 Bass kernel for the annular photonic transfer-matrix reflectance
sweep (W=2097152 wavelengths, L=6 layers), data-parallel over 8 NeuronCores.

Formulation (validated host-side to rel_l2 ~1.6e-3 vs the jax reference):
- Each shell's 2x2 transfer matrix entries are written as
      a = Ca(t)cosD - Sa(t)sinD,   b = (Cb sinD + Sb cosD)/p,
      c = (Cc sinD + Sc cosD)*p,   d = Cd cosD - Sd sinD,
  with t = omega*sqrt(eps), D = (r1-r0)*t, p = sqrt(eps).  The 8 smooth
  C/S product functions (Bessel amplitude/phase combinations at x0=t*r0,
  x1=t*r1) are fitted per shell by QUADRATICS in v = 1/t at build time
  (the reference's own y1 has a ~1e-3 jump at x=8, so degree>2 buys
  nothing).  Each quadratic is evaluated as A*(v+B)^2 + C: one ACT Square
  (free affine bias) + one DVE tensor_scalar.
- sinD/cosD: reduce t mod pi/c in ONE scalar_tensor_tensor (k<=14 so the
  single-constant reduction error is ~2e-6 rad), then ACT Sin with the
  *c fold in its free scale; the dropped (-1)^k sign scales the whole
  shell matrix and cancels in R = |num/den|^2.
- All smooth math runs in fp16 (DVE 2x/4x modes); t and the range
  reduction stay fp32.  Boundary C(z) factors are fitted the same way.
- mu == 1 per the spec; a numpy fallback guards other inputs.
"""
import numpy as np

import bass_rust
import concourse.bass as bass
import concourse.tile as tile
from concourse import mybir
from concourse.vector_clock import ScopedClock

F32 = mybir.dt.float32
F16 = mybir.dt.float16
AL = mybir.AluOpType
AF = mybir.ActivationFunctionType

W = 2097152
L = 6
NCORES = 8
P = 128
WS = W // NCORES          # 262144 elements per core
FT = WS // P              # 2048 free dim per core
FC = 1024                 # free-dim chunk per pass (2 passes)

PI = float(np.pi)
TWO_OVER_PI = 0.636619772
MAGIC = 8388608.0

# ---- NR Bessel coefficients (reference's own formulas, fp64 host eval) ------

J0_NUM = [-184.9052456, 77392.33017, -11214424.18, 651619640.7,
          -13362590354.0, 57568490574.0]
J0_DEN = [1.0, 267.8532712, 59272.64853, 9494680.718,
          1029532985.0, 57568490411.0]
J1_NUM = [-30.16036606, 15704.48260, -2972611.439, 242396853.1,
          -7895059235.0, 72362614232.0]
J1_DEN = [1.0, 376.9991397, 99447.43394, 18583304.74,
          2300535178.0, 144725228442.0]
Y0_NUM = [228.4622733, -86327.92757, 10879881.29, -512359803.6,
          7062834065.0, -2957821389.0]
Y0_DEN = [1.0, 226.1030244, 47447.26470, 7189466.438,
          745249964.8, 40076544269.0]
Y1_NUM = [8.511937935e4, -4.237922726e7, 7.349264551e9,
          -5.153438139e11, 1.275274390e13, -4.900604943e13]
Y1_DEN = [1.0, 3.549632885e3, 1.020426050e6, 2.245904002e8,
          3.733650367e10, 4.244419664e12, 2.499580570e14]
P0C = [0.2093887211e-6, -0.2073370639e-5, 0.2734510407e-4,
       -0.1098628627e-2, 1.0]
Q0C = [-0.934935152e-7, 0.7621095161e-6, -0.6911147651e-5,
       0.1430488765e-3, -0.1562499995e-1]
P1C = [-0.240337019e-6, 0.2457520174e-5, -0.3516396496e-4,
       0.183105e-2, 1.0]
Q1C = [0.105787412e-6, -0.88228987e-6, 0.8449199096e-5,
       -0.2002690873e-3, 0.04687499995]


def _hpoly(y, c):
    acc = np.full_like(y, c[0])
    for v in c[1:]:
        acc = acc * y + v
    return acc


def _j0(x):
    y = x * x
    small = _hpoly(y, J0_NUM) / _hpoly(y, J0_DEN)
    z = 8.0 / x
    y2 = z * z
    xx = x - 0.785398164
    big = np.sqrt(TWO_OVER_PI / x) * (np.cos(xx) * _hpoly(y2, P0C)
                                      - z * np.sin(xx) * _hpoly(y2, Q0C))
    return np.where(x < 8.0, small, big)


def _j1(x):
    y = x * x
    small = x * _hpoly(y, J1_NUM) / _hpoly(y, J1_DEN)
    z = 8.0 / x
    y2 = z * z
    xx = x - 2.356194491
    big = np.sqrt(TWO_OVER_PI / x) * (np.cos(xx) * _hpoly(y2, P1C)
                                      - z * np.sin(xx) * _hpoly(y2, Q1C))
    return np.where(x < 8.0, small, big)


def _y0(x):
    y = x * x
    small = _hpoly(y, Y0_NUM) / _hpoly(y, Y0_DEN) \
        + TWO_OVER_PI * _j0(x) * np.log(x)
    z = 8.0 / x
    y2 = z * z
    xx = x - 0.785398164
    big = np.sqrt(TWO_OVER_PI / x) * (np.sin(xx) * _hpoly(y2, P0C)
                                      + z * np.cos(xx) * _hpoly(y2, Q0C))
    return np.where(x < 8.0, small, big)


def _y1(x):
    y = x * x
    small = x * _hpoly(y, Y1_NUM) / _hpoly(y, Y1_DEN) \
        + TWO_OVER_PI * (_j1(x) * np.log(x) - 1.0 / x)
    z = 8.0 / x
    y2 = z * z
    xx = x - 2.356194491
    big = np.sqrt(TWO_OVER_PI / x) * (np.sin(xx) * _hpoly(y2, P1C)
                                      + z * np.cos(xx) * _hpoly(y2, Q1C))
    return np.where(x < 8.0, small, big)


def _mphi(x):
    amp = np.sqrt(PI * x / 2.0)
    j0n, y0n = _j0(x) * amp, _y0(x) * amp
    j1n, y1n = _j1(x) * amp, _y1(x) * amp
    psi = x - PI / 4
    m0 = np.hypot(j0n, y0n)
    ph0 = np.angle(np.exp(1j * (np.arctan2(y0n, j0n) - psi)))
    m1 = np.hypot(j1n, y1n)
    ph1 = np.angle(np.exp(1j * (np.arctan2(j1n, -y1n) - psi)))
    return m0, ph0, m1, ph1


def _shell_funcs(t, r0, r1):
    m0a, f0a, m1a, f1a = _mphi(t * r0)
    m0b, f0b, m1b, f1b = _mphi(t * r1)
    return (m1a * m0b * np.cos(f0b - f1a), m1a * m0b * np.sin(f0b - f1a),
            m0a * m0b * np.cos(f0b - f0a), m0a * m0b * np.sin(f0b - f0a),
            m1a * m1b * np.cos(f1b - f1a), m1a * m1b * np.sin(f1b - f1a),
            m0a * m1b * np.cos(f1b - f0a), m0a * m1b * np.sin(f1b - f0a))


def _bound_funcs(t, r):
    m0, ph0, m1, ph1 = _mphi(t * r)
    d = ph1 - ph0
    rm = m1 / m0
    return rm * np.sin(d), rm * np.cos(d)


LIN_TOL = 2.5e-3


def _fit_quad(f, lo, hi, n=3000):
    """Fit f on [lo,hi]: linear if it reaches LIN_TOL, else quadratic in
    square-form ("quad", A, B, C) for A*(v+B)^2 + C."""
    k = np.arange(n)
    x = lo + (hi - lo) * 0.5 * (1 - np.cos(np.pi * (k + 0.5) / n))
    y = f(x)
    ch1 = np.polynomial.chebyshev.Chebyshev.fit(x, y, 1, domain=[lo, hi])
    if np.abs(ch1(x) - y).max() < LIN_TOL:
        c1, c0 = 0.0, 0.0
        co = ch1.convert(kind=np.polynomial.Polynomial).coef
        c2 = co[0]
        c1 = co[1] if len(co) > 1 else 0.0
        return ("lin", float(c1), float(c2))
    ch = np.polynomial.chebyshev.Chebyshev.fit(x, y, 2, domain=[lo, hi])
    c2, c1, c0 = ch.convert(kind=np.polynomial.Polynomial).coef
    if abs(c0) < 1e-9:
        c0 = 1e-9 if c0 >= 0 else -1e-9
    return ("quad", float(c0), float(c1 / (2 * c0)),
            float(c2 - c1 * c1 / (4 * c0)))


def _build_fits(rho, tlo, thi):
    """rho: [L,2] float64; tlo/thi: per-layer t bounds. Returns dict."""
    fits = {}
    for l in range(1, L - 1):
        lo, hi = 1.0 / thi[l], 1.0 / tlo[l]
        r0, r1 = float(rho[l, 0]), float(rho[l, 1])
        for i, nm in enumerate(["Ca", "Sa", "Cb", "Sb", "Cc", "Sc",
                                "Cd", "Sd"]):
            fits[(l, nm)] = _fit_quad(
                lambda v, i=i: _shell_funcs(1.0 / v, r0, r1)[i], lo, hi)
    for (l, rr, pre) in [(0, float(rho[0, 1]), "b0"),
                         (L - 1, float(rho[L - 1, 0]), "b1")]:
        lo, hi = 1.0 / thi[l], 1.0 / tlo[l]
        for i, sfx in enumerate(["re", "im"]):
            fits[(l, pre + sfx)] = _fit_quad(
                lambda v, i=i: _bound_funcs(1.0 / v, rr)[i], lo, hi)
    return fits


# ---- walrus 1-sync-wait-per-instruction workaround --------------------------
_MAXW = 1


def _split_waits(nc):
    for f in nc.m.functions:
        for bb in f.blocks:
            arr = list(bb.instructions)
            out = []
            changed = False
            for mi in arr:
                si = mi.sync_info
                waits = list(si.on_wait) if si is not None and si.on_wait else []
                if len(waits) > _MAXW:
                    changed = True
                    upd = list(si.on_update) if si is not None and si.on_update \
                        else []
                    rest = waits[_MAXW:]
                    for i in range(0, len(rest), _MAXW):
                        ev = nc.engines[mi.engine].nop()
                        cur = nc.cur_bb.bb
                        cur.instructions = [
                            x for x in cur.instructions if x.name != ev.ins.name
                        ]
                        ev.ins.sync_info = bass_rust.SyncInfo(
                            on_wait=rest[i:i + _MAXW], on_update=[])
                        out.append(ev.ins)
                    mi.sync_info = bass_rust.SyncInfo(on_wait=waits[:_MAXW],
                                                      on_update=upd)
                out.append(mi)
            if changed:
                bb.instructions = out


def _patched_drain_and_barrier(self, tick_clock, wait_clock):
    nc = self.nc
    drain_inst = nc.sync.drain()
    wait_clock.add_sem_waits(
        drain_inst.ins, ScopedClock({None: tick_clock.global_clock})
    )
    nc.all_engine_barrier()
    assert self.sems is not None
    popped = nc._tile_sem_poison_stack.pop()
    assert popped is self._sem_poison
    nc.clear_and_free_semaphores(list(self.sems.allocated().values()))
    nc.all_engine_barrier()


tile.TileContext._drain_and_barrier = _patched_drain_and_barrier


def _register_const(nc, *values):
    for v in values:
        v = float(v)
        if (F32, v) in nc.const_aps.aps:
            continue
        t = nc.alloc_sbuf_tensor(f"const-f32-{v}", [128, 1], F32)
        nc.gpsimd.memset(t.ap(), v)
        nc.const_aps.aps[(F32, v)] = t.ap()
    nc.all_engine_barrier()


# ---- kernel emitter ---------------------------------------------------------

SHELL_FN = ["Ca", "Sa", "Cb", "Sb", "Cc", "Sc", "Cd", "Sd"]


def build(rho64, fits):
    nc = bass.Bass()
    biases = {float(np.float32(v[2])) for v in fits.values() if v[0] == "quad"}
    _register_const(nc, 0.0, PI / 2, *sorted(biases))

    om_d = nc.declare_dram_parameter("omega", [P, FT], F32, isOutput=False)
    ep_d = nc.declare_dram_parameter("eps", [L, P, FT], F32, isOutput=False)
    out_d = nc.declare_dram_parameter("out", [P, FT], F32, isOutput=True)

    with tile.TileContext(nc) as tc:
        with tc.tile_pool(name="work", bufs=1) as pool:
            n = [0]

            def mk(dt, tag, bufs):
                n[0] += 1
                return pool.tile([P, FC], dt, name=f"t{n[0]}", tag=tag,
                                 bufs=bufs)

            def w32(tag="g32", bufs=5):
                return mk(F32, tag, bufs)

            def w16(tag="g16", bufs=14):
                return mk(F16, tag, bufs)

            def act(out, in_, fn, bias=0.0, scale=1.0):
                nc.scalar.activation(out[:], in_[:], fn, float(bias),
                                     float(scale))
                return out

            def vts(out, a, s1, s2=None, op0="mult", op1="add"):
                if s2 is None:
                    nc.vector.tensor_scalar(out[:], a[:], float(s1), None,
                                            AL[op0])
                else:
                    nc.vector.tensor_scalar(out[:], a[:], float(s1),
                                            float(s2), AL[op0], AL[op1])
                return out

            def tt(out, a, b, op):
                nc.vector.tensor_tensor(out[:], a[:], b[:], AL[op])
                return out

            def stt(out, a, s, b, op0="mult", op1="add"):
                nc.vector.scalar_tensor_tensor(out[:], a[:], float(s), b[:],
                                               AL[op0], AL[op1])
                return out

            def poly16(fit, v16l):
                kind = fit[0]
                if kind == "lin":
                    return act(w16(), v16l, AF.Copy, fit[2], fit[1])
                _, A, B, C = fit
                B = float(np.float32(B))
                if abs(B) < 2.0:
                    q = act(w16(), v16l, AF.Square, B)
                    return vts(w16(), q, A, C)
                q = act(w32(), v16l, AF.Square, B)
                return act(w16(), q, AF.Copy, C, A)

            def chunk(ci, pre_tail=None):
                sl = slice(ci * FC, (ci + 1) * FC)
                omega = w32(tag="om", bufs=2)
                nc.sync.dma_start(omega[:], om_d[:, sl])
                ln_om = act(w32(tag="lnom", bufs=1), omega, AF.Ln)
                t16, v16, t_ = {}, {}, {}

                def layerA(l):
                    e = w32(tag="eps", bufs=2)
                    nc.sync.dma_start(e[:], ep_d[l, :, sl])
                    lne = act(w32(tag="sq", bufs=3), e, AF.Ln)
                    ln_t = stt(w32(tag="ln", bufs=2), lne, 0.5, ln_om)
                    t_[l] = act(w32(tag="t", bufs=4), ln_t, AF.Exp)
                    t16[l] = vts(w16(tag="t16", bufs=8), t_[l], 1.0)
                    v16[l] = act(w16(tag="v16", bufs=8), ln_t, AF.Exp,
                                 0.0, -1.0)

                def boundary(l, pre):
                    cre = poly16(fits[(l, pre + "re")], v16[l])
                    cim = poly16(fits[(l, pre + "im")], v16[l])
                    ur = tt(w16(tag="bnd", bufs=10), t16[l], cre, "mult")
                    ui = tt(w16(tag="bnd", bufs=10), t16[l], cim, "mult")
                    return ur, ui

                def shell(l):
                    r0 = float(rho64[l, 0])
                    r1 = float(rho64[l, 1])
                    c = float(np.float32(np.float64(r1) - np.float64(r0)))
                    cpi = float(np.float32(np.float64(c) / np.pi))
                    pic = float(np.float32(np.pi / np.float64(c)))
                    tr = vts(w32(), t_[l], cpi)
                    kf = vts(w32(), tr, MAGIC, MAGIC, "add", "subtract")
                    xr = stt(w32(), kf, -pic, t_[l])
                    SD = act(w16(tag="sdcd", bufs=8), xr, AF.Sin, 0.0, c)
                    CD = act(w16(tag="sdcd", bufs=8), xr, AF.Sin,
                             PI / 2, -c)
                    Pv = {nm: poly16(fits[(l, nm)], v16[l])
                          for nm in SHELL_FN}
                    TCa = tt(w16(), Pv["Ca"], CD, "mult")
                    TSa = tt(w16(), Pv["Sa"], SD, "mult")
                    TCb = tt(w16(), Pv["Cb"], SD, "mult")
                    TSb = tt(w16(), Pv["Sb"], CD, "mult")
                    TCc = tt(w16(), Pv["Cc"], SD, "mult")
                    TSc = tt(w16(), Pv["Sc"], CD, "mult")
                    TCd = tt(w16(), Pv["Cd"], CD, "mult")
                    TSd = tt(w16(), Pv["Sd"], SD, "mult")
                    a = tt(w16(tag="mm", bufs=16), TCa, TSa, "subtract")
                    beta = tt(w16(), TCb, TSb, "add")
                    gam = tt(w16(), TCc, TSc, "add")
                    d = tt(w16(tag="mm", bufs=16), TCd, TSd, "subtract")
                    b = tt(w16(tag="mm", bufs=16), beta, v16[l], "mult")
                    cc = tt(w16(tag="mm", bufs=16), gam, t16[l], "mult")
                    return a, b, cc, d

                def join(Mx, My):
                    a1, b1, c1, d1 = Mx
                    a2, b2, c2, d2 = My
                    A = tt(w16(tag="mm", bufs=16),
                           tt(w16(), a1, a2, "mult"),
                           tt(w16(), b1, c2, "mult"), "subtract")
                    Bq = tt(w16(tag="mm", bufs=16),
                            tt(w16(), a1, b2, "mult"),
                            tt(w16(), b1, d2, "mult"), "add")
                    C = tt(w16(tag="mm", bufs=16),
                           tt(w16(), c1, a2, "mult"),
                           tt(w16(), d1, c2, "mult"), "add")
                    D = tt(w16(tag="mm", bufs=16),
                           tt(w16(), d1, d2, "mult"),
                           tt(w16(), c1, b2, "mult"), "subtract")
                    return A, Bq, C, D

                layerA(0)
                layerA(L - 1)
                if pre_tail is not None:
                    pre_tail()
                layerA(1)
                u0 = boundary(0, "b0")
                u1 = boundary(L - 1, "b1")
                M = shell(1)
                layerA(2)
                layerA(3)
                M = join(M, shell(2))
                M = join(M, shell(3))
                layerA(4)
                M = join(M, shell(4))
                return dict(sl=sl, u0=u0, u1=u1, M=M)

            def tail(S):
                A, B, C, D = S["M"]
                ur0, ui0 = S["u0"]
                vr0, vi0 = S["u1"]
                Q = tt(w16(), ui0, B, "mult")
                er = tt(w16(), D, tt(w16(), ur0, B, "mult"), "add")
                T1 = tt(w16(), vi0, Q, "mult")
                T2 = tt(w16(), vr0, er, "mult")
                T3 = tt(w16(), vr0, Q, "mult")
                T4 = tt(w16(), vi0, er, "mult")
                aAr = tt(w16(), ur0, A, "mult")
                aAi = tt(w16(), ui0, A, "mult")
                b0 = tt(w16(), C, aAr, "subtract")
                b1 = tt(w16(), b0, T2, "add")
                Nr = tt(w16(), b1, T1, "subtract")
                Dr = tt(w16(), b1, T1, "add")
                c0_ = tt(w16(), aAi, T3, "subtract")
                Ni = tt(w16(), c0_, T4, "subtract")
                Di = tt(w16(), c0_, T4, "add")
                SN = tt(w32(), act(w32(), Nr, AF.Square),
                        act(w32(), Ni, AF.Square), "add")
                SDn = tt(w32(), act(w32(), Dr, AF.Square),
                         act(w32(), Di, AF.Square), "add")
                lnD = act(w32(), SDn, AF.Ln)
                rec = act(w32(), lnD, AF.Exp, 0.0, -1.0)
                R = tt(w32(), SN, rec, "mult")
                nc.sync.dma_start(out_d[:, S["sl"]], R[:])

            S0 = chunk(0)
            tail(S0)
            S1 = chunk(1)
            tail(S1)
    _split_waits(nc)
    return nc


# ---- host-side entry --------------------------------------------------------

_CACHE = {}
TRACE = False
LAST_RESULT = None


def _numpy_ref(omega, eps, mu, rho):
    """Exact reference math in numpy (fallback for mu != 1)."""
    omega = omega.astype(np.float64)
    eps = eps.astype(np.float64)
    mu = mu.astype(np.float64)
    rho = rho.astype(np.float64)
    k = omega[None, :] * np.sqrt(eps * mu)
    p = np.sqrt(eps / mu)

    def tmat(kl, pl, r0, r1):
        x0, x1 = kl * r0, kl * r1
        j_a, y_a = _j0(x0), _y0(x0)
        j_b, y_b = _j0(x1), _y0(x1)
        jd_a, yd_a = -_j1(x0), -_y1(x0)
        jd_b, yd_b = -_j1(x1), -_y1(x1)
        pref = (PI / 2) * x0
        m00 = pref * (yd_a * j_b - jd_a * y_b)
        m01 = (1j / pl) * pref * (j_a * y_b - y_a * j_b)
        m10 = (-1j * pl) * pref * (yd_a * jd_b - jd_a * yd_b)
        m11 = pref * (j_a * yd_b - y_a * jd_b)
        return m00 + 0j, m01, m10, m11 + 0j

    M00, M01, M10, M11 = tmat(k[1], p[1], rho[1, 0], rho[1, 1])
    for l in range(2, L - 1):
        a, b, c, d = tmat(k[l], p[l], rho[l, 0], rho[l, 1])
        M00, M01, M10, M11 = (M00 * a + M01 * c, M00 * b + M01 * d,
                              M10 * a + M11 * c, M10 * b + M11 * d)

    def cfacs(z):
        j0v, j1v, y0v, y1v = _j0(z), _j1(z), _y0(z), _y1(z)
        c1 = -(j1v + 1j * y1v) / (j0v + 1j * y0v)
        c2 = -(j1v - 1j * y1v) / (j0v - 1j * y0v)
        return c1, c2

    c0_1, c0_2 = cfacs(k[0] * rho[0, 1])
    _, c1_2 = cfacs(k[L - 1] * rho[L - 1, 0])
    p0, p1 = p[0], p[L - 1]
    num = M10 + 1j * p0 * c0_2 * M00 \
        - 1j * p1 * c1_2 * (M11 + 1j * p0 * c0_2 * M01)
    den = -1j * p0 * c0_1 * M00 - M10 \
        - 1j * p1 * c1_2 * (-1j * p0 * c0_1 * M01 - M11)
    r = num / den
    return (r * np.conj(r)).real.astype(np.float32)


def kernel(omega, eps, mu, rho):
    from concourse.bass_utils import run_bass_kernel_spmd

    omega = np.ascontiguousarray(omega, dtype=np.float32)
    eps = np.ascontiguousarray(eps, dtype=np.float32)
    mu = np.ascontiguousarray(mu, dtype=np.float32)
    rho = np.asarray(rho, dtype=np.float32)
    assert omega.shape == (W,) and eps.shape == (L, W)

    if not bool(np.all(mu == 1.0)):
        return _numpy_ref(omega, eps, mu, rho)

    rho64 = rho.astype(np.float64)
    om_lo, om_hi = float(omega.min()), float(omega.max())
    e_lo = eps.min(axis=1).astype(np.float64)
    e_hi = eps.max(axis=1).astype(np.float64)
    tlo = om_lo * np.sqrt(e_lo) * 0.999
    thi = om_hi * np.sqrt(e_hi) * 1.001

    key = (rho.tobytes(),
           tuple(np.round(tlo, 3).tolist()), tuple(np.round(thi, 3).tolist()))
    if key not in _CACHE:
        fits = _build_fits(rho64, tlo, thi)
        _CACHE[key] = build(rho64, fits)
    nc = _CACHE[key]

    in_maps = []
    for i in range(NCORES):
        sl = slice(i * WS, (i + 1) * WS)
        in_maps.append({"omega": omega[sl].reshape(P, FT),
                        "eps": eps[:, sl].reshape(L, P, FT)})

    res = run_bass_kernel_spmd(nc, in_maps, core_ids=list(range(NCORES)),
                               trace=TRACE)
    global LAST_RESULT
    LAST_RESULT = res
    out = np.empty((W,), dtype=np.float32)
    for i in range(NCORES):
        out[i * WS:(i + 1) * WS] = res.results[i]["out"].reshape(WS)
    return out
